# revision 9
# baseline (speedup 1.0000x reference)
"""ChunkedLinearAttention Trainium2 kernel — 8-core SPMD.

Sharding: core c -> batch b = c//2, head-half hh = c%2 (8 of 16 heads).
Each core computes qkv projection for its heads, chunked local attention +
cross-chunk linear term, and a row-sharded out-projection producing a partial
[4096, 1024] output; host sums the two half partials per batch element.

All matmuls in bf16 (fp32 accumulate in PSUM).  Layouts:
  xT    [1024, 4096]  x[b] transposed (host-side), bf16
  qkT   [cols, tok]   computed on PE: lhsT=Wqk tile, rhs=xT tile
  v     [tok, vcols]  computed on PE: lhsT=xT tile, rhs=Wv
  per head-pair: qT/kT [128(2 heads x 64 dims), 8 chunks, 64 tok]
  scores S [128(2 heads x 64 q), 8, 64 k] via per-chunk matmuls packed with
  tile_position (head A rows 0-63 / head B rows 64-127)
  out_localT [128(2 heads x 64 dims), 512 tok] accumulated in PSUM, with the
  cross term added via a [K=8 chunks] matmul against cum_v.
"""

import sys

if "/opt/trn_rl_repo" not in sys.path:
    sys.path.insert(0, "/opt/trn_rl_repo")

import numpy as np
import ml_dtypes

import concourse.bacc as bacc
import concourse.tile as tile
import concourse.mybir as mybir
from concourse.bass_utils import run_bass_kernel_spmd

F32 = mybir.dt.float32
BF16 = mybir.dt.bfloat16
AFT = mybir.ActivationFunctionType

DIM, H, D, CS = 1024, 16, 64, 64
SCALE = D ** -0.5
B, N = 4, 4096
NBLK, TB = 8, 512          # token blocks
NC_CHUNKS = 8              # chunks per block
HPC = 8                    # heads per core
NPAIR = 4                  # head pairs per core
N_CORES = 8

_cache = {}


def _build():
    nc = bacc.Bacc("TRN2", target_bir_lowering=False, debug=False,
                   num_devices=N_CORES)

    # ---- DRAM I/O -------------------------------------------------------
    xT_d = nc.dram_tensor("xT", [DIM, N], BF16, kind="ExternalInput")
    wqk_d = nc.dram_tensor("wqk", [DIM, 1024], BF16, kind="ExternalInput")
    wv_d = nc.dram_tensor("wv", [DIM, 512], BF16, kind="ExternalInput")
    wout_d = nc.dram_tensor("wout", [512, DIM], BF16, kind="ExternalInput")
    ident_d = nc.dram_tensor("ident", [128, 128], BF16, kind="ExternalInput")
    maskqk_d = nc.dram_tensor("maskqk", [128, 512], F32, kind="ExternalInput")
    mean_d = nc.dram_tensor("meanm", [128, 32], BF16, kind="ExternalInput")
    triexc_d = nc.dram_tensor("triexc", [8, 8], BF16, kind="ExternalInput")
    ones18_d = nc.dram_tensor("ones18", [1, 8], BF16, kind="ExternalInput")
    ones81_d = nc.dram_tensor("ones81", [8, 1], BF16, kind="ExternalInput")
    ones11_d = nc.dram_tensor("ones11", [1, 1], BF16, kind="ExternalInput")
    onesD_d = nc.dram_tensor("onesD", [128, 1], BF16, kind="ExternalInput")
    bdmask_d = nc.dram_tensor("bdmask", [8, 1024], F32, kind="ExternalInput")
    out_d = nc.dram_tensor("out", [N, DIM], F32, kind="ExternalOutput")

    with tile.TileContext(nc) as tc:
        with (
            tc.tile_pool(name="const", bufs=1) as cpool,
            tc.tile_pool(name="persist", bufs=1) as ppool,
            tc.tile_pool(name="work", bufs=2) as wpool,
            tc.tile_pool(name="ps", bufs=7, space="PSUM") as ps,
        ):
            # ---- constants / weights into SBUF --------------------------
            ident = cpool.tile([128, 128], BF16, name="ident")
            nc.sync.dma_start(ident[:], ident_d[:])
            maskqk = cpool.tile([128, 8, 64], F32, name="maskqk")
            nc.sync.dma_start(maskqk[:], maskqk_d.rearrange("p (c k) -> p c k", c=8))
            meanm = cpool.tile([128, 32], BF16, name="meanm")
            nc.sync.dma_start(meanm[:], mean_d[:])
            triexc = cpool.tile([8, 8], BF16, name="triexc")
            nc.sync.dma_start(triexc[:], triexc_d[:])
            ones18 = cpool.tile([1, 8], BF16, name="ones18")
            nc.sync.dma_start(ones18[:], ones18_d[:])
            ones81 = cpool.tile([8, 1], BF16, name="ones81")
            nc.sync.dma_start(ones81[:], ones81_d[:])
            ones11 = cpool.tile([1, 1], BF16, name="ones11")
            nc.sync.dma_start(ones11[:], ones11_d[:])
            onesD = cpool.tile([128, 1], BF16, name="onesD")
            nc.sync.dma_start(onesD[:], onesD_d[:])
            bdmask = cpool.tile([8, 1024], F32, name="bdmask")
            nc.sync.dma_start(bdmask[:], bdmask_d[:])

            xT = []
            for i in range(8):
                t = ppool.tile([128, N], BF16, name=f"xT{i}", tag=f"xT{i}")
                nc.sync.dma_start(t[:], xT_d[i * 128:(i + 1) * 128, :])
                xT.append(t)
            wqk = []
            for i in range(8):
                t = ppool.tile([128, 1024], BF16, name=f"wqk{i}", tag=f"wqk{i}")
                nc.sync.dma_start(t[:], wqk_d[i * 128:(i + 1) * 128, :])
                wqk.append(t)
            wv = []
            for i in range(8):
                t = ppool.tile([128, 512], BF16, name=f"wv{i}", tag=f"wv{i}")
                nc.sync.dma_start(t[:], wv_d[i * 128:(i + 1) * 128, :])
                wv.append(t)
            wout = []
            for p in range(NPAIR):
                t = ppool.tile([128, 1024], BF16, name=f"wout{p}", tag=f"wout{p}")
                nc.sync.dma_start(t[:], wout_d[p * 128:(p + 1) * 128, :])
                wout.append(t)

            # ---- cross-block running state ------------------------------
            runv = ppool.tile([1, 512], BF16, name="runv", tag="runv")
            nc.vector.memset(runv[:], 0.0)
            runkT = []
            for p in range(NPAIR):
                t = ppool.tile([128, 1], F32, name=f"runkT{p}", tag=f"runkT{p}")
                nc.vector.memset(t[:], 0.0)
                runkT.append(t)

            # ---- main loop over token blocks ----------------------------
            for t in range(NBLK):
                tok0 = t * TB

                # ---- qkT projection: 8 M-tiles (4 q pairs, 4 k pairs) ----
                qT = [None] * NPAIR
                kT = [None] * NPAIR
                for mt in range(8):
                    acc = ps.tile([128, 512], F32, name=f"qk_ps_{t}_{mt}", tag="m")
                    for i in range(8):
                        nc.tensor.matmul(
                            acc[:], wqk[i][:, mt * 128:(mt + 1) * 128],
                            xT[i][:, tok0:tok0 + TB],
                            start=(i == 0), stop=(i == 7))
                    dst = wpool.tile([128, 8, 64], BF16, name=f"qkT_{t}_{mt}",
                                     tag=f"qkT{mt}")
                    nc.scalar.copy(dst[:], acc[:].rearrange("p (c k) -> p c k", c=8))
                    if mt < 4:
                        qT[mt] = dst
                    else:
                        kT[mt - 4] = dst

                # ---- v projection: 4 token tiles -------------------------
                v_sb = [None] * 4
                v_lo = [None] * 4  # odd chunk shifted to partitions 0-63
                chunkv_ps = ps.tile([8, 512], F32, name=f"cv_ps_{t}", tag="m")
                for vt in range(4):
                    acc = ps.tile([128, 512], F32, name=f"v_ps_{t}_{vt}", tag="m")
                    for i in range(8):
                        nc.tensor.matmul(
                            acc[:], xT[i][:, tok0 + vt * 128:tok0 + (vt + 1) * 128],
                            wv[i][:], start=(i == 0), stop=(i == 7))
                    dst = wpool.tile([128, 512], BF16, name=f"v_{t}_{vt}", tag=f"v{vt}")
                    nc.vector.tensor_copy(dst[:], acc[:])
                    v_sb[vt] = dst
                    dst2 = wpool.tile([64, 512], BF16, name=f"vlo_{t}_{vt}",
                                      tag=f"vlo{vt}")
                    nc.vector.tensor_copy(dst2[:], dst[64:128, :])
                    v_lo[vt] = dst2
                    # chunk means (x 0.5/64 folded into meanm)
                    nc.tensor.matmul(chunkv_ps[:], meanm[:, vt * 8:(vt + 1) * 8],
                                     dst[:], start=(vt == 0), stop=(vt == 3))

                chunkv = wpool.tile([8, 512], BF16, name=f"cv_{t}", tag="cv")
                nc.vector.tensor_copy(chunkv[:], chunkv_ps[:])

                # exclusive cumsum over chunks + running carry
                cumv_ps = ps.tile([8, 512], F32, name=f"cumv_ps_{t}", tag="m")
                nc.tensor.matmul(cumv_ps[:], triexc[:], chunkv[:],
                                 start=True, stop=False)
                nc.tensor.matmul(cumv_ps[:], ones18[:], runv[:],
                                 start=False, stop=True)
                cumv = wpool.tile([8, 512], BF16, name=f"cumv_{t}", tag="cumv")
                nc.vector.tensor_copy(cumv[:], cumv_ps[:])
                # running += sum_c chunk_v
                runp = ps.tile([1, 512], F32, name=f"runp_ps_{t}", tag="m")
                nc.tensor.matmul(runp[:], ones81[:], chunkv[:],
                                 start=True, stop=False)
                nc.tensor.matmul(runp[:], ones11[:], runv[:],
                                 start=False, stop=True)
                nc.vector.tensor_copy(runv[:], runp[:])

                # ---- per head-pair attention ----------------------------
                for p in range(NPAIR):
                    # cum_kT: segmented reduce + shifted adds (f32)
                    ckT = wpool.tile([128, 8], F32, name=f"ckT_{t}_{p}", tag="ckT")
                    nc.vector.tensor_reduce(ckT[:], kT[p][:],
                                            axis=mybir.AxisListType.X,
                                            op=mybir.AluOpType.add)
                    t1 = wpool.tile([128, 8], F32, name=f"t1_{t}_{p}", tag="t1")
                    nc.vector.tensor_copy(t1[:, 0:1], ckT[:, 0:1])
                    nc.vector.tensor_add(t1[:, 1:8], ckT[:, 0:7], ckT[:, 1:8])
                    t2 = wpool.tile([128, 8], F32, name=f"t2_{t}_{p}", tag="t2")
                    nc.vector.tensor_copy(t2[:, 0:2], t1[:, 0:2])
                    nc.vector.tensor_add(t2[:, 2:8], t1[:, 0:6], t1[:, 2:8])
                    incl = wpool.tile([128, 8], F32, name=f"incl_{t}_{p}", tag="incl")
                    nc.vector.tensor_copy(incl[:, 0:4], t2[:, 0:4])
                    nc.vector.tensor_add(incl[:, 4:8], t2[:, 0:4], t2[:, 4:8])
                    cumkT = wpool.tile([128, 8], F32, name=f"cumkT_{t}_{p}",
                                       tag="cumkT")
                    nc.vector.tensor_copy(cumkT[:, 0:1], runkT[p][:, 0:1])
                    nc.vector.tensor_add(cumkT[:, 1:8], incl[:, 0:7],
                                         runkT[p][:, 0:1].broadcast_to([128, 7]))
                    nc.vector.tensor_add(runkT[p][:], runkT[p][:], incl[:, 7:8])

                    # CKT broadcast to [128, 8, 64] bf16
                    CKT = wpool.tile([128, 8, 64], BF16, name=f"CKT_{t}_{p}",
                                     tag="CKT")
                    nc.vector.tensor_copy(CKT[:], cumkT[:].broadcast_to([128, 8, 64]))

                    # scores: 16 packed chunk matmuls
                    s8 = ps.tile([128, 512], F32, name=f"s8_{t}_{p}", tag="m")
                    for c in range(8):
                        nc.tensor.matmul(
                            s8[0:64, c * 64:(c + 1) * 64],
                            qT[p][0:64, c, :], kT[p][0:64, c, :],
                            start=True, stop=True, tile_position=(0, 0))
                        nc.tensor.matmul(
                            s8[64:128, c * 64:(c + 1) * 64],
                            qT[p][64:128, c, :], kT[p][64:128, c, :],
                            start=True, stop=True, tile_position=(64, 64))

                    # softmax (no max-subtraction; masked post-exp)
                    E = wpool.tile([128, 8, 64], F32, name=f"E_{t}_{p}", tag="E")
                    nc.scalar.activation(E[:], s8[:].rearrange("p (c k) -> p c k", c=8),
                                         AFT.Exp)
                    Em = wpool.tile([128, 8, 64], F32, name=f"Em_{t}_{p}", tag="Em")
                    nc.vector.tensor_mul(Em[:], E[:], maskqk[:])
                    denom = wpool.tile([128, 8], F32, name=f"den_{t}_{p}", tag="den")
                    nc.vector.tensor_reduce(denom[:], Em[:],
                                            axis=mybir.AxisListType.X,
                                            op=mybir.AluOpType.add)
                    recip = wpool.tile([128, 8], F32, name=f"rec_{t}_{p}", tag="rec")
                    nc.vector.reciprocal(recip[:], denom[:])
                    attn = wpool.tile([128, 8, 64], BF16, name=f"attn_{t}_{p}",
                                      tag="attn")
                    nc.vector.tensor_mul(attn[:], Em[:],
                                         recip[:].broadcast_to([128, 8, 64]))

                    # transpose attn per chunk: [128q, 64k] -> [64k, 128q]
                    et1 = ps.tile([64, 512], BF16, name=f"et1_{t}_{p}", tag="m")
                    et2 = ps.tile([64, 512], BF16, name=f"et2_{t}_{p}", tag="m")
                    for c in range(8):
                        dst_ps = et1 if c < 4 else et2
                        nc.tensor.transpose(
                            dst_ps[:, (c % 4) * 128:(c % 4 + 1) * 128],
                            attn[:, c, :], ident[:])
                    ET = wpool.tile([64, 8, 128], BF16, name=f"ET_{t}_{p}", tag="ET")
                    nc.scalar.copy(ET[:, 0:4, :],
                                   et1[:].rearrange("p (c q) -> p c q", c=4))
                    nc.scalar.copy(ET[:, 4:8, :],
                                   et2[:].rearrange("p (c q) -> p c q", c=4))

                    # cross term: cross_pre = colsum(qT * CKT) / 64
                    P = wpool.tile([128, 8, 64], BF16, name=f"P_{t}_{p}", tag="P")
                    nc.vector.tensor_mul(P[:], qT[p][:], CKT[:])
                    crps = ps.tile([64, 512], F32, name=f"cr_{t}_{p}", tag="m")
                    nc.tensor.matmul(crps[0:1, :], onesD[0:64, :],
                                     P[0:64, :, :].rearrange("p c k -> p (c k)"),
                                     start=True, stop=True, tile_position=(0, 0))
                    nc.tensor.matmul(crps[32:33, :], onesD[64:128, :],
                                     P[64:128, :, :].rearrange("p c k -> p (c k)"),
                                     start=True, stop=True, tile_position=(64, 32))
                    cross = wpool.tile([1, 1024], BF16, name=f"cross_{t}_{p}",
                                       tag="cross")
                    nc.scalar.activation(cross[0:1, 0:512], crps[0:1, :], AFT.Sigmoid)
                    nc.scalar.activation(cross[0:1, 512:1024], crps[32:33, :],
                                         AFT.Sigmoid)
                    # W8 = blockdiag(cross): outer product then mask
                    w8ps_a = ps.tile([8, 512], F32, name=f"w8a_{t}_{p}", tag="m")
                    w8ps_b = ps.tile([8, 512], F32, name=f"w8b_{t}_{p}", tag="m")
                    nc.tensor.matmul(w8ps_a[:], ones18[:], cross[0:1, 0:512],
                                     start=True, stop=True)
                    nc.tensor.matmul(w8ps_b[:], ones18[:], cross[0:1, 512:1024],
                                     start=True, stop=True)
                    W8 = wpool.tile([8, 1024], BF16, name=f"W8_{t}_{p}", tag="W8")
                    nc.vector.tensor_mul(W8[:, 0:512], w8ps_a[:], bdmask[:, 0:512])
                    nc.vector.tensor_mul(W8[:, 512:1024], w8ps_b[:],
                                         bdmask[:, 512:1024])

                    # out_localT + cross term, accumulated in PSUM
                    o_ps = ps.tile([128, 512], F32, name=f"o_{t}_{p}", tag="m")
                    for c in range(8):
                        vt_, lo = c // 2, (c % 2)
                        vA = (v_sb[vt_] if lo == 0 else v_lo[vt_])
                        nc.tensor.matmul(
                            o_ps[0:64, c * 64:(c + 1) * 64],
                            vA[0:64, 2 * p * 64:(2 * p + 1) * 64],
                            ET[:, c, 0:64],
                            start=(c == 0), stop=False, tile_position=(0, 0),
                            skip_group_check=True)
                        nc.tensor.matmul(
                            o_ps[64:128, c * 64:(c + 1) * 64],
                            vA[0:64, (2 * p + 1) * 64:(2 * p + 2) * 64],
                            ET[:, c, 64:128],
                            start=(c == 0), stop=False, tile_position=(0, 64),
                            skip_group_check=True)
                    nc.tensor.matmul(o_ps[0:64, :], cumv[:, 2 * p * 64:(2 * p + 1) * 64],
                                     W8[:, 0:512], start=False, stop=True,
                                     tile_position=(0, 0), skip_group_check=True)
                    nc.tensor.matmul(o_ps[64:128, :],
                                     cumv[:, (2 * p + 1) * 64:(2 * p + 2) * 64],
                                     W8[:, 512:1024], start=False, stop=True,
                                     tile_position=(0, 64), skip_group_check=True)
                    oT = wpool.tile([128, 512], BF16, name=f"oT_{t}_{p}", tag=f"oT{p}")
                    nc.vector.tensor_copy(oT[:], o_ps[:])
                    if p == 0:
                        oTs = [oT]
                    else:
                        oTs.append(oT)

                # ---- out projection -------------------------------------
                for nt in range(2):
                    for tt in range(4):
                        fo = ps.tile([128, 512], F32, name=f"fo_{t}_{nt}_{tt}",
                                     tag="m")
                        for p in range(NPAIR):
                            nc.tensor.matmul(
                                fo[:], oTs[p][:, tt * 128:(tt + 1) * 128],
                                wout[p][:, nt * 512:(nt + 1) * 512],
                                start=(p == 0), stop=(p == 3))
                        fs = wpool.tile([128, 512], F32, name=f"fs_{t}_{nt}_{tt}",
                                        tag="fs")
                        nc.scalar.copy(fs[:], fo[:])
                        nc.sync.dma_start(
                            out_d[tok0 + tt * 128:tok0 + (tt + 1) * 128,
                                  nt * 512:(nt + 1) * 512], fs[:])

    nc.compile()
    return nc


def _consts():
    ident = np.eye(128, dtype=ml_dtypes.bfloat16)
    # causal in-chunk mask: row p (q = p % 64), col j valid if j <= q
    q = np.arange(128)[:, None] % 64
    j = np.arange(64)[None, :]
    maskqk = np.tile((j <= q).astype(np.float32), (1, 8))
    # chunk-mean matrices with 0.5 (cross factor) / 64 (mean) folded in
    meanm = np.zeros((128, 32), dtype=np.float32)
    for vt in range(4):
        meanm[0:64, vt * 8 + 2 * vt] = 0.5 / 64
        meanm[64:128, vt * 8 + 2 * vt + 1] = 0.5 / 64
    triexc = np.triu(np.ones((8, 8), np.float32), 1)  # [c', c] = 1 if c' < c
    ones18 = np.ones((1, 8), np.float32)
    ones81 = np.ones((8, 1), np.float32)
    ones11 = np.ones((1, 1), np.float32)
    onesD = np.full((128, 1), 1.0 / 64, np.float32)
    # block-diag mask for W8: [8, 1024]; cols 0-511 head A, 512-1023 head B
    c_ = np.arange(8)[:, None]
    col = np.arange(512)[None, :]
    bd = (col // 64 == c_).astype(np.float32)
    bdmask = np.concatenate([bd, bd], axis=1)
    bf = ml_dtypes.bfloat16
    return {
        "ident": ident,
        "maskqk": maskqk,
        "meanm": meanm.astype(bf),
        "triexc": triexc.astype(bf),
        "ones18": ones18.astype(bf),
        "ones81": ones81.astype(bf),
        "ones11": ones11.astype(bf),
        "onesD": onesD.astype(bf),
        "bdmask": bdmask,
    }


def _in_maps(x, W_qkv, W_out):
    bf = ml_dtypes.bfloat16
    consts = _consts()
    maps = []
    for c in range(N_CORES):
        b, hh = c // 2, c % 2
        heads = list(range(hh * HPC, (hh + 1) * HPC))
        xT = np.ascontiguousarray(x[b].T).astype(bf)
        qcols = np.concatenate(
            [W_qkv[:, 0 * DIM + h * D:(0 * DIM) + (h + 1) * D] for h in heads], axis=1)
        kcols = np.concatenate(
            [W_qkv[:, 1 * DIM + h * D:1 * DIM + (h + 1) * D] for h in heads], axis=1)
        vcols = np.concatenate(
            [W_qkv[:, 2 * DIM + h * D:2 * DIM + (h + 1) * D] for h in heads], axis=1)
        wqk = np.concatenate([qcols * SCALE, kcols], axis=1).astype(bf)
        wv = vcols.astype(bf)
        wout = np.concatenate([W_out[h * D:(h + 1) * D, :] for h in heads],
                              axis=0).astype(bf)
        m = {"xT": xT, "wqk": np.ascontiguousarray(wqk),
             "wv": np.ascontiguousarray(wv), "wout": np.ascontiguousarray(wout)}
        m.update(consts)
        maps.append(m)
    return maps


def kernel(x, W_qkv, W_out, _trace=False):
    if "nc" not in _cache:
        _cache["nc"] = _build()
    nc = _cache["nc"]
    maps = _in_maps(np.asarray(x, np.float32), np.asarray(W_qkv, np.float32),
                    np.asarray(W_out, np.float32))
    res = run_bass_kernel_spmd(nc, maps, core_ids=list(range(N_CORES)),
                               trace=_trace)
    _cache["last_result"] = res
    out = np.empty((B, N, DIM), np.float32)
    for b in range(B):
        out[b] = res.results[2 * b]["out"] + res.results[2 * b + 1]["out"]
    return out


# revision 13
# speedup vs baseline: 1.0868x; 1.0868x over previous
"""ChunkedLinearAttention Trainium2 kernel — 8-core SPMD.

Sharding: core c -> batch b = c//2, head-half hh = c%2 (8 of 16 heads).
Each core computes qkv projection for its heads, chunked local attention +
cross-chunk linear term, and a row-sharded out-projection producing a partial
[4096, 1024] output; host sums the two half partials per batch element.

All matmuls in bf16 (fp32 accumulate in PSUM).  Layouts:
  xT    [1024, 4096]  x[b] transposed (host-side), bf16
  qkT   [cols, tok]   computed on PE: lhsT=Wqk tile, rhs=xT tile
  v     [tok, vcols]  computed on PE: lhsT=xT tile, rhs=Wv
  per head-pair: qT/kT [128(2 heads x 64 dims), 8 chunks, 64 tok]
  scores S [128(2 heads x 64 q), 8, 64 k] via per-chunk matmuls packed with
  tile_position (head A rows 0-63 / head B rows 64-127)
  out_localT [128(2 heads x 64 dims), 512 tok] accumulated in PSUM, with the
  cross term added via a [K=8 chunks] matmul against cum_v.
"""

import sys

if "/opt/trn_rl_repo" not in sys.path:
    sys.path.insert(0, "/opt/trn_rl_repo")

import numpy as np
import ml_dtypes

import concourse.bacc as bacc
import concourse.tile as tile
import concourse.mybir as mybir
from concourse.bass_utils import run_bass_kernel_spmd

F32 = mybir.dt.float32
BF16 = mybir.dt.bfloat16
AFT = mybir.ActivationFunctionType

DIM, H, D, CS = 1024, 16, 64, 64
SCALE = D ** -0.5
B, N = 4, 4096
NBLK, TB = 8, 512          # token blocks
NC_CHUNKS = 8              # chunks per block
HPC = 8                    # heads per core
NPAIR = 4                  # head pairs per core
N_CORES = 8

_cache = {}


def _build():
    nc = bacc.Bacc("TRN2", target_bir_lowering=False, debug=False,
                   num_devices=N_CORES)

    # ---- DRAM I/O -------------------------------------------------------
    xT_d = nc.dram_tensor("xT", [DIM, N], BF16, kind="ExternalInput")
    wqk_d = nc.dram_tensor("wqk", [DIM, 1024], BF16, kind="ExternalInput")
    wv_d = nc.dram_tensor("wv", [DIM, 512], BF16, kind="ExternalInput")
    wout_d = nc.dram_tensor("wout", [512, DIM], BF16, kind="ExternalInput")
    ident_d = nc.dram_tensor("ident", [128, 128], BF16, kind="ExternalInput")
    maskqk_d = nc.dram_tensor("maskqk", [128, 512], F32, kind="ExternalInput")
    mean_d = nc.dram_tensor("meanm", [128, 32], BF16, kind="ExternalInput")
    triexc_d = nc.dram_tensor("triexc", [8, 8], BF16, kind="ExternalInput")
    ones18_d = nc.dram_tensor("ones18", [64, 8], BF16, kind="ExternalInput")
    ones81_d = nc.dram_tensor("ones81", [8, 1], BF16, kind="ExternalInput")
    ones11_d = nc.dram_tensor("ones11", [1, 1], BF16, kind="ExternalInput")
    onesD_d = nc.dram_tensor("onesD", [128, 1], BF16, kind="ExternalInput")
    bdmask_d = nc.dram_tensor("bdmask", [8, 1024], F32, kind="ExternalInput")
    out_d = nc.dram_tensor("out", [N, DIM], F32, kind="ExternalOutput")

    with tile.TileContext(nc) as tc:
        with (
            tc.tile_pool(name="const", bufs=1) as cpool,
            tc.tile_pool(name="persist", bufs=1) as ppool,
            tc.tile_pool(name="work", bufs=2) as wpool,
            tc.tile_pool(name="psq", bufs=3, space="PSUM") as psq,
            tc.tile_pool(name="psa", bufs=4, space="PSUM") as psa,
        ):
            # ---- constants / weights into SBUF --------------------------
            ident = cpool.tile([128, 128], BF16, name="ident")
            nc.sync.dma_start(ident[:], ident_d[:])
            maskqk = cpool.tile([128, 8, 64], F32, name="maskqk")
            nc.sync.dma_start(maskqk[:], maskqk_d.rearrange("p (c k) -> p c k", c=8))
            meanm = cpool.tile([128, 32], BF16, name="meanm")
            nc.sync.dma_start(meanm[:], mean_d[:])
            triexc = cpool.tile([8, 8], BF16, name="triexc")
            nc.sync.dma_start(triexc[:], triexc_d[:])
            ones18 = cpool.tile([64, 8], BF16, name="ones18")
            nc.sync.dma_start(ones18[:], ones18_d[:])
            ones81 = cpool.tile([8, 1], BF16, name="ones81")
            nc.sync.dma_start(ones81[:], ones81_d[:])
            ones11 = cpool.tile([1, 1], BF16, name="ones11")
            nc.sync.dma_start(ones11[:], ones11_d[:])
            onesD = cpool.tile([128, 1], BF16, name="onesD")
            nc.sync.dma_start(onesD[:], onesD_d[:])
            bdmask = cpool.tile([8, 1024], F32, name="bdmask")
            nc.sync.dma_start(bdmask[:], bdmask_d[:])

            xT = []
            for i in range(8):
                t = ppool.tile([128, N], BF16, name=f"xT{i}", tag=f"xT{i}")
                nc.sync.dma_start(t[:], xT_d[i * 128:(i + 1) * 128, :])
                xT.append(t)
            wqk = []
            for i in range(8):
                t = ppool.tile([128, 1024], BF16, name=f"wqk{i}", tag=f"wqk{i}")
                nc.sync.dma_start(t[:], wqk_d[i * 128:(i + 1) * 128, :])
                wqk.append(t)
            wv = []
            for i in range(8):
                t = ppool.tile([128, 512], BF16, name=f"wv{i}", tag=f"wv{i}")
                nc.sync.dma_start(t[:], wv_d[i * 128:(i + 1) * 128, :])
                wv.append(t)
            wout = []
            for p in range(NPAIR):
                t = ppool.tile([128, 1024], BF16, name=f"wout{p}", tag=f"wout{p}")
                nc.sync.dma_start(t[:], wout_d[p * 128:(p + 1) * 128, :])
                wout.append(t)

            # ---- cross-block running state ------------------------------
            runv = ppool.tile([1, 512], BF16, name="runv", tag="runv")
            nc.vector.memset(runv[:], 0.0)
            runkT = []
            for p in range(NPAIR):
                t = ppool.tile([128, 1], F32, name=f"runkT{p}", tag=f"runkT{p}")
                nc.vector.memset(t[:], 0.0)
                runkT.append(t)

            # ---- main loop over token blocks ----------------------------
            for t in range(NBLK):
                tok0 = t * TB

                # ---- qkT projection: 8 M-tiles (4 q pairs, 4 k pairs) ----
                qT = [None] * NPAIR
                kT = [None] * NPAIR
                for mt in range(8):
                    acc = psq.tile([128, 512], F32, name=f"qk_ps_{t}_{mt}", tag="m")
                    for i in range(8):
                        nc.tensor.matmul(
                            acc[:], wqk[i][:, mt * 128:(mt + 1) * 128],
                            xT[i][:, tok0:tok0 + TB],
                            start=(i == 0), stop=(i == 7))
                    dst = wpool.tile([128, 8, 64], BF16, name=f"qkT_{t}_{mt}",
                                     tag=f"qkT{mt}")
                    nc.scalar.copy(dst[:], acc[:].rearrange("p (c k) -> p c k", c=8))
                    if mt < 4:
                        qT[mt] = dst
                    else:
                        kT[mt - 4] = dst

                # ---- v projection: 4 token tiles -------------------------
                v_sb = [None] * 4
                v_lo = [None] * 4  # odd chunk shifted to partitions 0-63
                chunkv_ps = psq.tile([8, 512], F32, name=f"cv_ps_{t}", tag="m")
                for vt in range(4):
                    acc = psq.tile([128, 512], F32, name=f"v_ps_{t}_{vt}", tag="m")
                    for i in range(8):
                        nc.tensor.matmul(
                            acc[:], xT[i][:, tok0 + vt * 128:tok0 + (vt + 1) * 128],
                            wv[i][:], start=(i == 0), stop=(i == 7))
                    dst = wpool.tile([128, 512], BF16, name=f"v_{t}_{vt}", tag=f"v{vt}")
                    nc.vector.tensor_copy(dst[:], acc[:])
                    v_sb[vt] = dst
                    dst2 = wpool.tile([64, 512], BF16, name=f"vlo_{t}_{vt}",
                                      tag=f"vlo{vt}")
                    nc.gpsimd.tensor_copy(dst2[:], dst[64:128, :])
                    v_lo[vt] = dst2
                    # chunk means (x 0.5/64 folded into meanm)
                    nc.tensor.matmul(chunkv_ps[:], meanm[:, vt * 8:(vt + 1) * 8],
                                     dst[:], start=(vt == 0), stop=(vt == 3))

                chunkv = wpool.tile([8, 512], BF16, name=f"cv_{t}", tag="cv")
                nc.vector.tensor_copy(chunkv[:], chunkv_ps[:])

                # exclusive cumsum over chunks + running carry
                cumv_ps = psq.tile([8, 512], F32, name=f"cumv_ps_{t}", tag="m")
                nc.tensor.matmul(cumv_ps[:], triexc[:], chunkv[:],
                                 start=True, stop=False)
                nc.tensor.matmul(cumv_ps[:], ones18[0:1, :], runv[:],
                                 start=False, stop=True)
                cumv = wpool.tile([8, 512], BF16, name=f"cumv_{t}", tag="cumv")
                nc.vector.tensor_copy(cumv[:], cumv_ps[:])
                # running += sum_c chunk_v
                runp = psq.tile([1, 512], F32, name=f"runp_ps_{t}", tag="m")
                nc.tensor.matmul(runp[:], ones81[:], chunkv[:],
                                 start=True, stop=False)
                nc.tensor.matmul(runp[:], ones11[:], runv[:],
                                 start=False, stop=True)
                nc.vector.tensor_copy(runv[:], runp[:])

                # ---- per head-pair attention ----------------------------
                for p in range(NPAIR):
                    # cum_kT: segmented reduce + shifted adds (f32)
                    ckT = wpool.tile([128, 8], F32, name=f"ckT_{t}_{p}", tag="ckT")
                    nc.vector.tensor_reduce(ckT[:], kT[p][:],
                                            axis=mybir.AxisListType.X,
                                            op=mybir.AluOpType.add)
                    t1 = wpool.tile([128, 8], F32, name=f"t1_{t}_{p}", tag="t1")
                    nc.vector.tensor_copy(t1[:, 0:1], ckT[:, 0:1])
                    nc.vector.tensor_add(t1[:, 1:8], ckT[:, 0:7], ckT[:, 1:8])
                    t2 = wpool.tile([128, 8], F32, name=f"t2_{t}_{p}", tag="t2")
                    nc.vector.tensor_copy(t2[:, 0:2], t1[:, 0:2])
                    nc.vector.tensor_add(t2[:, 2:8], t1[:, 0:6], t1[:, 2:8])
                    incl = wpool.tile([128, 8], F32, name=f"incl_{t}_{p}", tag="incl")
                    nc.vector.tensor_copy(incl[:, 0:4], t2[:, 0:4])
                    nc.vector.tensor_add(incl[:, 4:8], t2[:, 0:4], t2[:, 4:8])
                    cumkT = wpool.tile([128, 8], F32, name=f"cumkT_{t}_{p}",
                                       tag="cumkT")
                    nc.vector.tensor_copy(cumkT[:, 0:1], runkT[p][:, 0:1])
                    nc.vector.tensor_add(cumkT[:, 1:8], incl[:, 0:7],
                                         runkT[p][:, 0:1].broadcast_to([128, 7]))
                    nc.vector.tensor_add(runkT[p][:], runkT[p][:], incl[:, 7:8])

                    # CKT broadcast to [128, 8, 64] bf16
                    CKT = wpool.tile([128, 8, 64], BF16, name=f"CKT_{t}_{p}",
                                     tag="CKT")
                    nc.vector.tensor_copy(CKT[:], cumkT[:].broadcast_to([128, 8, 64]))

                    # scores: 16 packed chunk matmuls
                    s8 = psa.tile([128, 512], F32, name=f"s8_{t}_{p}", tag="m")
                    for c in range(8):
                        nc.tensor.matmul(
                            s8[0:64, c * 64:(c + 1) * 64],
                            qT[p][0:64, c, :], kT[p][0:64, c, :],
                            start=True, stop=True, tile_position=(0, 0))
                        nc.tensor.matmul(
                            s8[64:128, c * 64:(c + 1) * 64],
                            qT[p][64:128, c, :], kT[p][64:128, c, :],
                            start=True, stop=True, tile_position=(64, 64))

                    # softmax (no max-subtraction; masked post-exp)
                    E = wpool.tile([128, 8, 64], F32, name=f"E_{t}_{p}", tag="E")
                    nc.scalar.activation(E[:], s8[:].rearrange("p (c k) -> p c k", c=8),
                                         AFT.Exp)
                    Em = wpool.tile([128, 8, 64], F32, name=f"Em_{t}_{p}", tag="Em")
                    nc.vector.tensor_mul(Em[:], E[:], maskqk[:])
                    denom = wpool.tile([128, 8], F32, name=f"den_{t}_{p}", tag="den")
                    nc.vector.tensor_reduce(denom[:], Em[:],
                                            axis=mybir.AxisListType.X,
                                            op=mybir.AluOpType.add)
                    recip = wpool.tile([128, 8], F32, name=f"rec_{t}_{p}", tag="rec")
                    nc.vector.reciprocal(recip[:], denom[:])
                    attn = wpool.tile([128, 8, 64], BF16, name=f"attn_{t}_{p}",
                                      tag="attn")
                    nc.vector.tensor_mul(attn[:], Em[:],
                                         recip[:].broadcast_to([128, 8, 64]))

                    # transpose attn per chunk: [128q, 64k] -> [64k, 128q]
                    et1 = psa.tile([64, 512], BF16, name=f"et1_{t}_{p}", tag="m")
                    et2 = psa.tile([64, 512], BF16, name=f"et2_{t}_{p}", tag="m")
                    for c in range(8):
                        dst_ps = et1 if c < 4 else et2
                        nc.tensor.transpose(
                            dst_ps[:, (c % 4) * 128:(c % 4 + 1) * 128],
                            attn[:, c, :], ident[:])
                    ET = wpool.tile([64, 8, 128], BF16, name=f"ET_{t}_{p}", tag="ET")
                    nc.scalar.copy(ET[:, 0:4, :],
                                   et1[:].rearrange("p (c q) -> p c q", c=4))
                    nc.scalar.copy(ET[:, 4:8, :],
                                   et2[:].rearrange("p (c q) -> p c q", c=4))

                    # cross term: cross_pre = colsum(qT * CKT) / 64
                    P = wpool.tile([128, 8, 64], BF16, name=f"P_{t}_{p}", tag="P")
                    nc.vector.tensor_mul(P[:], qT[p][:], CKT[:])
                    crps = psa.tile([64, 512], F32, name=f"cr_{t}_{p}", tag="m")
                    nc.tensor.matmul(crps[0:1, :], onesD[0:64, :],
                                     P[0:64, :, :].rearrange("p c k -> p (c k)"),
                                     start=True, stop=True, tile_position=(0, 0))
                    nc.tensor.matmul(crps[32:33, :], onesD[64:128, :],
                                     P[64:128, :, :].rearrange("p c k -> p (c k)"),
                                     start=True, stop=True, tile_position=(64, 32))
                    cross = wpool.tile([33, 512], BF16, name=f"cross_{t}_{p}",
                                       tag="cross")
                    nc.scalar.activation(cross[:], crps[0:33, :], AFT.Sigmoid)
                    # W8 = blockdiag(cross): outer product then mask
                    w8ps_a = psa.tile([8, 512], F32, name=f"w8a_{t}_{p}", tag="m")
                    w8ps_b = psa.tile([8, 512], F32, name=f"w8b_{t}_{p}", tag="m")
                    nc.tensor.matmul(w8ps_a[:], ones18[0:1, :], cross[0:1, :],
                                     start=True, stop=True)
                    nc.tensor.matmul(w8ps_b[:], ones18[32:33, :], cross[32:33, :],
                                     start=True, stop=True)
                    W8 = wpool.tile([8, 1024], BF16, name=f"W8_{t}_{p}", tag="W8")
                    nc.vector.tensor_mul(W8[:, 0:512], w8ps_a[:], bdmask[:, 0:512])
                    nc.vector.tensor_mul(W8[:, 512:1024], w8ps_b[:],
                                         bdmask[:, 512:1024])

                    # out_localT + cross term, accumulated in PSUM
                    o_ps = psa.tile([128, 512], F32, name=f"o_{t}_{p}", tag="m")
                    for c in range(8):
                        vt_, lo = c // 2, (c % 2)
                        vA = (v_sb[vt_] if lo == 0 else v_lo[vt_])
                        nc.tensor.matmul(
                            o_ps[0:64, c * 64:(c + 1) * 64],
                            vA[0:64, 2 * p * 64:(2 * p + 1) * 64],
                            ET[:, c, 0:64],
                            start=(c == 0), stop=False, tile_position=(0, 0),
                            skip_group_check=True)
                        nc.tensor.matmul(
                            o_ps[64:128, c * 64:(c + 1) * 64],
                            vA[0:64, (2 * p + 1) * 64:(2 * p + 2) * 64],
                            ET[:, c, 64:128],
                            start=(c == 0), stop=False, tile_position=(0, 64),
                            skip_group_check=True)
                    nc.tensor.matmul(o_ps[0:64, :], cumv[:, 2 * p * 64:(2 * p + 1) * 64],
                                     W8[:, 0:512], start=False, stop=True,
                                     tile_position=(0, 0), skip_group_check=True)
                    nc.tensor.matmul(o_ps[64:128, :],
                                     cumv[:, (2 * p + 1) * 64:(2 * p + 2) * 64],
                                     W8[:, 512:1024], start=False, stop=True,
                                     tile_position=(0, 64), skip_group_check=True)
                    oT = wpool.tile([128, 512], BF16, name=f"oT_{t}_{p}", tag=f"oT{p}")
                    nc.vector.tensor_copy(oT[:], o_ps[:])
                    if p == 0:
                        oTs = [oT]
                    else:
                        oTs.append(oT)

                # ---- out projection -------------------------------------
                for nt in range(2):
                    for tt in range(4):
                        fo = psq.tile([128, 512], F32, name=f"fo_{t}_{nt}_{tt}",
                                     tag="m")
                        for p in range(NPAIR):
                            nc.tensor.matmul(
                                fo[:], oTs[p][:, tt * 128:(tt + 1) * 128],
                                wout[p][:, nt * 512:(nt + 1) * 512],
                                start=(p == 0), stop=(p == 3))
                        fs = wpool.tile([128, 512], F32, name=f"fs_{t}_{nt}_{tt}",
                                        tag="fs")
                        nc.scalar.copy(fs[:], fo[:])
                        nc.sync.dma_start(
                            out_d[tok0 + tt * 128:tok0 + (tt + 1) * 128,
                                  nt * 512:(nt + 1) * 512], fs[:])

    nc.compile()
    return nc


def _consts():
    ident = np.eye(128, dtype=ml_dtypes.bfloat16)
    # causal in-chunk mask: row p (q = p % 64), col j valid if j <= q
    q = np.arange(128)[:, None] % 64
    j = np.arange(64)[None, :]
    maskqk = np.tile((j <= q).astype(np.float32), (1, 8))
    # chunk-mean matrices with 0.5 (cross factor) / 64 (mean) folded in
    meanm = np.zeros((128, 32), dtype=np.float32)
    for vt in range(4):
        meanm[0:64, vt * 8 + 2 * vt] = 0.5 / 64
        meanm[64:128, vt * 8 + 2 * vt + 1] = 0.5 / 64
    triexc = np.triu(np.ones((8, 8), np.float32), 1)  # [c', c] = 1 if c' < c
    ones18 = np.ones((64, 8), np.float32)
    ones81 = np.ones((8, 1), np.float32)
    ones11 = np.ones((1, 1), np.float32)
    onesD = np.full((128, 1), 1.0 / 64, np.float32)
    # block-diag mask for W8: [8, 1024]; cols 0-511 head A, 512-1023 head B
    c_ = np.arange(8)[:, None]
    col = np.arange(512)[None, :]
    bd = (col // 64 == c_).astype(np.float32)
    bdmask = np.concatenate([bd, bd], axis=1)
    bf = ml_dtypes.bfloat16
    return {
        "ident": ident,
        "maskqk": maskqk,
        "meanm": meanm.astype(bf),
        "triexc": triexc.astype(bf),
        "ones18": ones18.astype(bf),
        "ones81": ones81.astype(bf),
        "ones11": ones11.astype(bf),
        "onesD": onesD.astype(bf),
        "bdmask": bdmask,
    }


def _in_maps(x, W_qkv, W_out):
    bf = ml_dtypes.bfloat16
    consts = _consts()
    maps = []
    for c in range(N_CORES):
        b, hh = c // 2, c % 2
        heads = list(range(hh * HPC, (hh + 1) * HPC))
        xT = np.ascontiguousarray(x[b].T).astype(bf)
        qcols = np.concatenate(
            [W_qkv[:, 0 * DIM + h * D:(0 * DIM) + (h + 1) * D] for h in heads], axis=1)
        kcols = np.concatenate(
            [W_qkv[:, 1 * DIM + h * D:1 * DIM + (h + 1) * D] for h in heads], axis=1)
        vcols = np.concatenate(
            [W_qkv[:, 2 * DIM + h * D:2 * DIM + (h + 1) * D] for h in heads], axis=1)
        wqk = np.concatenate([qcols * SCALE, kcols], axis=1).astype(bf)
        wv = vcols.astype(bf)
        wout = np.concatenate([W_out[h * D:(h + 1) * D, :] for h in heads],
                              axis=0).astype(bf)
        m = {"xT": xT, "wqk": np.ascontiguousarray(wqk),
             "wv": np.ascontiguousarray(wv), "wout": np.ascontiguousarray(wout)}
        m.update(consts)
        maps.append(m)
    return maps


def kernel(x, W_qkv, W_out, _trace=False):
    if "nc" not in _cache:
        _cache["nc"] = _build()
    nc = _cache["nc"]
    maps = _in_maps(np.asarray(x, np.float32), np.asarray(W_qkv, np.float32),
                    np.asarray(W_out, np.float32))
    res = run_bass_kernel_spmd(nc, maps, core_ids=list(range(N_CORES)),
                               trace=_trace)
    _cache["last_result"] = res
    out = np.empty((B, N, DIM), np.float32)
    for b in range(B):
        out[b] = res.results[2 * b]["out"] + res.results[2 * b + 1]["out"]
    return out


# revision 18
# speedup vs baseline: 1.2351x; 1.1364x over previous
"""ChunkedLinearAttention Trainium2 kernel — 8-core SPMD.

Sharding: core c -> batch b = c//2, head-half hh = c%2 (8 of 16 heads).
Each core computes qkv projection for its heads, chunked local attention +
cross-chunk linear term, and a row-sharded out-projection producing a partial
[4096, 1024] output; host sums the two half partials per batch element.

All matmuls in bf16 (fp32 accumulate in PSUM).  Layouts:
  xT    [1024, 4096]  x[b] transposed (host-side), bf16
  qkT   [cols, tok]   computed on PE: lhsT=Wqk tile, rhs=xT tile
  v     [tok, vcols]  computed on PE: lhsT=xT tile, rhs=Wv
  per head-pair: qT/kT [128(2 heads x 64 dims), 8 chunks, 64 tok]
  scores S [128(2 heads x 64 q), 8, 64 k] via per-chunk matmuls packed with
  tile_position (head A rows 0-63 / head B rows 64-127)
  out_localT [128(2 heads x 64 dims), 512 tok] accumulated in PSUM, with the
  cross term added via a [K=8 chunks] matmul against cum_v.
"""

import sys

if "/opt/trn_rl_repo" not in sys.path:
    sys.path.insert(0, "/opt/trn_rl_repo")

import numpy as np
import ml_dtypes

import concourse.bacc as bacc
import concourse.tile as tile
import concourse.mybir as mybir
from concourse.bass_utils import run_bass_kernel_spmd

F32 = mybir.dt.float32
BF16 = mybir.dt.bfloat16
AFT = mybir.ActivationFunctionType

DIM, H, D, CS = 1024, 16, 64, 64
SCALE = D ** -0.5
B, N = 4, 4096
NBLK, TB = 8, 512          # token blocks
NC_CHUNKS = 8              # chunks per block
HPC = 8                    # heads per core
NPAIR = 4                  # head pairs per core
N_CORES = 8

_cache = {}


def _build():
    nc = bacc.Bacc("TRN2", target_bir_lowering=False, debug=False,
                   num_devices=N_CORES)

    # ---- DRAM I/O -------------------------------------------------------
    xT_d = nc.dram_tensor("xT", [DIM, N], BF16, kind="ExternalInput")
    wqk_d = nc.dram_tensor("wqk", [DIM, 1024], BF16, kind="ExternalInput")
    wv_d = nc.dram_tensor("wv", [DIM, 512], BF16, kind="ExternalInput")
    wout_d = nc.dram_tensor("wout", [512, DIM], BF16, kind="ExternalInput")
    ident_d = nc.dram_tensor("ident", [128, 128], BF16, kind="ExternalInput")
    maskqk_d = nc.dram_tensor("maskqk", [128, 2048], BF16, kind="ExternalInput")
    mean_d = nc.dram_tensor("meanm", [128, 32], BF16, kind="ExternalInput")
    triexc_d = nc.dram_tensor("triexc", [8, 8], BF16, kind="ExternalInput")
    ones18_d = nc.dram_tensor("ones18", [128, 8], BF16, kind="ExternalInput")
    ones81_d = nc.dram_tensor("ones81", [8, 1], BF16, kind="ExternalInput")
    ones11_d = nc.dram_tensor("ones11", [1, 1], BF16, kind="ExternalInput")
    onesD_d = nc.dram_tensor("onesD", [128, 1], BF16, kind="ExternalInput")
    bdmask_d = nc.dram_tensor("bdmask", [8, 1024], F32, kind="ExternalInput")
    out_d = nc.dram_tensor("out", [N, DIM], F32, kind="ExternalOutput")

    with tile.TileContext(nc) as tc:
        with (
            tc.tile_pool(name="const", bufs=1) as cpool,
            tc.tile_pool(name="persist", bufs=1) as ppool,
            tc.tile_pool(name="work", bufs=2) as wpool,
            tc.tile_pool(name="psq", bufs=3, space="PSUM") as psq,
            tc.tile_pool(name="psa", bufs=3, space="PSUM") as psa,
            tc.tile_pool(name="psw", bufs=1, space="PSUM") as psw,
        ):
            # ---- constants / weights into SBUF --------------------------
            ident = cpool.tile([128, 128], BF16, name="ident")
            nc.sync.dma_start(ident[:], ident_d[:])
            maskqk = cpool.tile([128, 32, 64], BF16, name="maskqk")
            nc.sync.dma_start(maskqk[:], maskqk_d.rearrange("p (c k) -> p c k", c=32))
            meanm = cpool.tile([128, 32], BF16, name="meanm")
            nc.sync.dma_start(meanm[:], mean_d[:])
            triexc = cpool.tile([8, 8], BF16, name="triexc")
            nc.sync.dma_start(triexc[:], triexc_d[:])
            ones18 = cpool.tile([128, 8], BF16, name="ones18")
            nc.sync.dma_start(ones18[:], ones18_d[:])
            ones81 = cpool.tile([8, 1], BF16, name="ones81")
            nc.sync.dma_start(ones81[:], ones81_d[:])
            ones11 = cpool.tile([1, 1], BF16, name="ones11")
            nc.sync.dma_start(ones11[:], ones11_d[:])
            onesD = cpool.tile([128, 1], BF16, name="onesD")
            nc.sync.dma_start(onesD[:], onesD_d[:])
            bdmask = cpool.tile([8, 1024], F32, name="bdmask")
            nc.sync.dma_start(bdmask[:], bdmask_d[:])

            xT = []
            for i in range(8):
                t = ppool.tile([128, N], BF16, name=f"xT{i}", tag=f"xT{i}")
                nc.sync.dma_start(t[:], xT_d[i * 128:(i + 1) * 128, :])
                xT.append(t)
            wqk = []
            for i in range(8):
                t = ppool.tile([128, 1024], BF16, name=f"wqk{i}", tag=f"wqk{i}")
                nc.sync.dma_start(t[:], wqk_d[i * 128:(i + 1) * 128, :])
                wqk.append(t)
            wv = []
            for i in range(8):
                t = ppool.tile([128, 512], BF16, name=f"wv{i}", tag=f"wv{i}")
                nc.sync.dma_start(t[:], wv_d[i * 128:(i + 1) * 128, :])
                wv.append(t)
            wout = []
            for p in range(NPAIR):
                t = ppool.tile([128, 1024], BF16, name=f"wout{p}", tag=f"wout{p}")
                nc.sync.dma_start(t[:], wout_d[p * 128:(p + 1) * 128, :])
                wout.append(t)

            # ---- cross-block running state ------------------------------
            runv = ppool.tile([1, 512], BF16, name="runv", tag="runv")
            nc.vector.memset(runv[:], 0.0)
            runkT = ppool.tile([128, 4], F32, name="runkT", tag="runkT")
            nc.vector.memset(runkT[:], 0.0)

            # ---- main loop over token blocks ----------------------------
            for t in range(NBLK):
                tok0 = t * TB

                # ---- qkT projection: 8 M-tiles (4 q pairs, 4 k pairs) ----
                qT_all = wpool.tile([128, 4, 8, 64], BF16, name=f"qT_all_{t}",
                                    tag="qT_all")
                kT_all = wpool.tile([128, 4, 8, 64], BF16, name=f"kT_all_{t}",
                                    tag="kT_all")
                for mt in range(8):
                    acc = psq.tile([128, 512], F32, name=f"qk_ps_{t}_{mt}", tag="m")
                    for i in range(8):
                        nc.tensor.matmul(
                            acc[:], wqk[i][:, mt * 128:(mt + 1) * 128],
                            xT[i][:, tok0:tok0 + TB],
                            start=(i == 0), stop=(i == 7))
                    dst = (qT_all if mt < 4 else kT_all)
                    nc.scalar.copy(dst[:, mt % 4, :, :],
                                   acc[:].rearrange("p (c k) -> p c k", c=8))

                # ---- v projection: 4 token tiles -------------------------
                v_sb = [None] * 4
                v_lo = [None] * 4  # odd chunk shifted to partitions 0-63
                chunkv_ps = psq.tile([8, 512], F32, name=f"cv_ps_{t}", tag="m")
                for vt in range(4):
                    acc = psq.tile([128, 512], F32, name=f"v_ps_{t}_{vt}", tag="m")
                    for i in range(8):
                        nc.tensor.matmul(
                            acc[:], xT[i][:, tok0 + vt * 128:tok0 + (vt + 1) * 128],
                            wv[i][:], start=(i == 0), stop=(i == 7))
                    dst = wpool.tile([128, 512], BF16, name=f"v_{t}_{vt}", tag=f"v{vt}")
                    nc.vector.tensor_copy(dst[:], acc[:])
                    v_sb[vt] = dst
                    dst2 = wpool.tile([64, 512], BF16, name=f"vlo_{t}_{vt}",
                                      tag=f"vlo{vt}")
                    nc.gpsimd.tensor_copy(dst2[:], dst[64:128, :])
                    v_lo[vt] = dst2
                    # chunk means (x 0.5/64 folded into meanm)
                    nc.tensor.matmul(chunkv_ps[:], meanm[:, vt * 8:(vt + 1) * 8],
                                     dst[:], start=(vt == 0), stop=(vt == 3))

                chunkv = wpool.tile([8, 512], BF16, name=f"cv_{t}", tag="cv")
                nc.vector.tensor_copy(chunkv[:], chunkv_ps[:])

                # exclusive cumsum over chunks + running carry
                cumv_ps = psq.tile([8, 512], F32, name=f"cumv_ps_{t}", tag="m")
                nc.tensor.matmul(cumv_ps[:], triexc[:], chunkv[:],
                                 start=True, stop=False)
                nc.tensor.matmul(cumv_ps[:], ones18[0:1, :], runv[:],
                                 start=False, stop=True)
                cumv = wpool.tile([8, 512], BF16, name=f"cumv_{t}", tag="cumv")
                nc.vector.tensor_copy(cumv[:], cumv_ps[:])
                # running += sum_c chunk_v
                runp = psq.tile([1, 512], F32, name=f"runp_ps_{t}", tag="m")
                nc.tensor.matmul(runp[:], ones81[:], chunkv[:],
                                 start=True, stop=False)
                nc.tensor.matmul(runp[:], ones11[:], runv[:],
                                 start=False, stop=True)
                nc.vector.tensor_copy(runv[:], runp[:])

                # ---- merged softmax/cross pipeline over all 4 pairs -----
                # cum_kT for all pairs: one reduce + merged shift-add cumsum
                ckT = wpool.tile([128, 4, 8], F32, name=f"ckT_{t}", tag="ckT")
                nc.vector.tensor_reduce(ckT[:], kT_all[:],
                                        axis=mybir.AxisListType.X,
                                        op=mybir.AluOpType.add)
                t1 = wpool.tile([128, 4, 8], F32, name=f"t1_{t}", tag="t1")
                nc.vector.tensor_copy(t1[:, :, 0:1], ckT[:, :, 0:1])
                nc.vector.tensor_add(t1[:, :, 1:8], ckT[:, :, 0:7], ckT[:, :, 1:8])
                t2 = wpool.tile([128, 4, 8], F32, name=f"t2_{t}", tag="t2")
                nc.vector.tensor_copy(t2[:, :, 0:2], t1[:, :, 0:2])
                nc.vector.tensor_add(t2[:, :, 2:8], t1[:, :, 0:6], t1[:, :, 2:8])
                incl = wpool.tile([128, 4, 8], F32, name=f"incl_{t}", tag="incl")
                nc.vector.tensor_copy(incl[:, :, 0:4], t2[:, :, 0:4])
                nc.vector.tensor_add(incl[:, :, 4:8], t2[:, :, 0:4], t2[:, :, 4:8])
                cumkT = wpool.tile([128, 4, 8], F32, name=f"cumkT_{t}", tag="cumkT")
                nc.vector.tensor_copy(cumkT[:, :, 0:1],
                                      runkT[:].broadcast_to([128, 4, 1]))
                nc.vector.tensor_add(cumkT[:, :, 1:8], incl[:, :, 0:7],
                                     runkT[:].broadcast_to([128, 4, 7]))
                nc.vector.tensor_add(runkT[:], runkT[:],
                                     incl[:, :, 7:8].rearrange("p a b -> p (a b)"))

                CKT = wpool.tile([128, 32, 64], BF16, name=f"CKT_{t}", tag="CKT")
                nc.vector.tensor_copy(
                    CKT[:],
                    cumkT[:].rearrange("p a b -> p (a b)").broadcast_to([128, 32, 64]))
                P = wpool.tile([128, 32, 64], BF16, name=f"P_{t}", tag="P")
                nc.vector.tensor_mul(P[:], qT_all[:].rearrange("p a c k -> p (a c) k"),
                                     CKT[:])

                # cross_pre column sums (1/64 folded into onesD), 2 psum tiles
                crps = [None, None]
                for g in range(2):
                    crps[g] = psa.tile([128, 512], F32, name=f"cr_{t}_{g}", tag="m")
                    for pp in range(2):
                        p = 2 * g + pp
                        nc.tensor.matmul(
                            crps[g][64 * pp:64 * pp + 1, :], onesD[0:64, :],
                            P[0:64, 8 * p:8 * (p + 1), :].rearrange("p c k -> p (c k)"),
                            start=True, stop=True, tile_position=(0, 64 * pp),
                            skip_group_check=True)
                        nc.tensor.matmul(
                            crps[g][64 * pp + 32:64 * pp + 33, :], onesD[64:128, :],
                            P[64:128, 8 * p:8 * (p + 1), :].rearrange("p c k -> p (c k)"),
                            start=True, stop=True, tile_position=(64, 64 * pp + 32),
                            skip_group_check=True)
                cross = wpool.tile([128, 512], BF16, name=f"cross_{t}", tag="cross")
                nc.scalar.activation(cross[0:97, :], crps[0][0:97, :], AFT.Sigmoid)
                # second group lands in rows 0..97 of crps[1]; write rows 0..97
                cross2 = wpool.tile([128, 512], BF16, name=f"cross2_{t}", tag="cross2")
                nc.scalar.activation(cross2[0:97, :], crps[1][0:97, :], AFT.Sigmoid)

                # scores for all pairs (16 MMs each) + merged softmax
                s8s = [None] * NPAIR
                for p in range(NPAIR):
                    s8 = psa.tile([128, 512], F32, name=f"s8_{t}_{p}", tag="m")
                    for c in range(8):
                        nc.tensor.matmul(
                            s8[0:64, c * 64:(c + 1) * 64],
                            qT_all[0:64, p, c, :], kT_all[0:64, p, c, :],
                            start=True, stop=True, tile_position=(0, 0))
                        nc.tensor.matmul(
                            s8[64:128, c * 64:(c + 1) * 64],
                            qT_all[64:128, p, c, :], kT_all[64:128, p, c, :],
                            start=True, stop=True, tile_position=(64, 64))
                    s8s[p] = s8

                E = wpool.tile([128, 32, 64], BF16, name=f"E_{t}", tag="E")
                for p in range(NPAIR):
                    nc.scalar.activation(E[:, 8 * p:8 * (p + 1), :],
                                         s8s[p][:].rearrange("p (c k) -> p c k", c=8),
                                         AFT.Exp)
                nc.vector.tensor_mul(E[:], E[:], maskqk[:])
                denom = wpool.tile([128, 32], F32, name=f"den_{t}", tag="den")
                nc.vector.tensor_reduce(denom[:], E[:], axis=mybir.AxisListType.X,
                                        op=mybir.AluOpType.add)
                recip = wpool.tile([128, 32], F32, name=f"rec_{t}", tag="rec")
                nc.vector.reciprocal(recip[:], denom[:])
                attn = wpool.tile([128, 32, 64], BF16, name=f"attn_{t}", tag="attn")
                nc.vector.tensor_mul(attn[:], E[:],
                                     recip[:].broadcast_to([128, 32, 64]))

                oTs = []
                for p in range(NPAIR):
                    # transpose attn per chunk: [128q, 64k] -> [64k, 128q]
                    et1 = psa.tile([64, 512], BF16, name=f"et1_{t}_{p}", tag="m")
                    et2 = psa.tile([64, 512], BF16, name=f"et2_{t}_{p}", tag="m")
                    for c in range(8):
                        dst_ps = et1 if c < 4 else et2
                        nc.tensor.transpose(
                            dst_ps[:, (c % 4) * 128:(c % 4 + 1) * 128],
                            attn[:, 8 * p + c, :], ident[:])
                    ET = wpool.tile([64, 8, 128], BF16, name=f"ET_{t}_{p}", tag="ET")
                    nc.scalar.copy(ET[:, 0:4, :],
                                   et1[:].rearrange("p (c q) -> p c q", c=4))
                    nc.scalar.copy(ET[:, 4:8, :],
                                   et2[:].rearrange("p (c q) -> p c q", c=4))

                    # W8 = blockdiag(sigmoid(cross_pre))
                    crt = cross if p < 2 else cross2
                    rowA = 64 * (p % 2)
                    w8ps = psw.tile([8, 1024], F32, name=f"w8_{t}_{p}", tag="w8")
                    nc.tensor.matmul(w8ps[:, 0:512], ones18[rowA:rowA + 1, :],
                                     crt[rowA:rowA + 1, :], start=True, stop=True,
                                     tile_position=(rowA, 0))
                    nc.tensor.matmul(w8ps[:, 512:1024],
                                     ones18[rowA + 32:rowA + 33, :],
                                     crt[rowA + 32:rowA + 33, :], start=True,
                                     stop=True, tile_position=(rowA + 32, 0))
                    W8 = wpool.tile([8, 1024], BF16, name=f"W8_{t}_{p}", tag="W8")
                    nc.vector.tensor_mul(W8[:], w8ps[:], bdmask[:])

                    # out_localT + cross term, accumulated in PSUM
                    o_ps = psa.tile([128, 512], F32, name=f"o_{t}_{p}", tag="m")
                    for c in range(8):
                        vt_, lo = c // 2, (c % 2)
                        vA = (v_sb[vt_] if lo == 0 else v_lo[vt_])
                        nc.tensor.matmul(
                            o_ps[0:64, c * 64:(c + 1) * 64],
                            vA[0:64, 2 * p * 64:(2 * p + 1) * 64],
                            ET[:, c, 0:64],
                            start=(c == 0), stop=False, tile_position=(0, 0),
                            skip_group_check=True)
                        nc.tensor.matmul(
                            o_ps[64:128, c * 64:(c + 1) * 64],
                            vA[0:64, (2 * p + 1) * 64:(2 * p + 2) * 64],
                            ET[:, c, 64:128],
                            start=(c == 0), stop=False, tile_position=(0, 64),
                            skip_group_check=True)
                    nc.tensor.matmul(o_ps[0:64, :], cumv[:, 2 * p * 64:(2 * p + 1) * 64],
                                     W8[:, 0:512], start=False, stop=True,
                                     tile_position=(0, 0), skip_group_check=True)
                    nc.tensor.matmul(o_ps[64:128, :],
                                     cumv[:, (2 * p + 1) * 64:(2 * p + 2) * 64],
                                     W8[:, 512:1024], start=False, stop=True,
                                     tile_position=(0, 64), skip_group_check=True)
                    oT = wpool.tile([128, 512], BF16, name=f"oT_{t}_{p}", tag=f"oT{p}")
                    nc.vector.tensor_copy(oT[:], o_ps[:])
                    oTs.append(oT)

                # ---- out projection -------------------------------------
                for nt in range(2):
                    for tt in range(4):
                        fo = psq.tile([128, 512], F32, name=f"fo_{t}_{nt}_{tt}",
                                     tag="m")
                        for p in range(NPAIR):
                            nc.tensor.matmul(
                                fo[:], oTs[p][:, tt * 128:(tt + 1) * 128],
                                wout[p][:, nt * 512:(nt + 1) * 512],
                                start=(p == 0), stop=(p == 3))
                        fs = wpool.tile([128, 512], F32, name=f"fs_{t}_{nt}_{tt}",
                                        tag="fs")
                        nc.scalar.copy(fs[:], fo[:])
                        nc.sync.dma_start(
                            out_d[tok0 + tt * 128:tok0 + (tt + 1) * 128,
                                  nt * 512:(nt + 1) * 512], fs[:])

    nc.compile()
    return nc


def _consts():
    ident = np.eye(128, dtype=ml_dtypes.bfloat16)
    # causal in-chunk mask: row p (q = p % 64), col j valid if j <= q
    q = np.arange(128)[:, None] % 64
    j = np.arange(64)[None, :]
    maskqk = np.tile((j <= q).astype(np.float32), (1, 32)).astype(ml_dtypes.bfloat16)
    # chunk-mean matrices with 0.5 (cross factor) / 64 (mean) folded in
    meanm = np.zeros((128, 32), dtype=np.float32)
    for vt in range(4):
        meanm[0:64, vt * 8 + 2 * vt] = 0.5 / 64
        meanm[64:128, vt * 8 + 2 * vt + 1] = 0.5 / 64
    triexc = np.triu(np.ones((8, 8), np.float32), 1)  # [c', c] = 1 if c' < c
    ones18 = np.ones((128, 8), np.float32)
    ones81 = np.ones((8, 1), np.float32)
    ones11 = np.ones((1, 1), np.float32)
    onesD = np.full((128, 1), 1.0 / 64, np.float32)
    # block-diag mask for W8: [8, 1024]; cols 0-511 head A, 512-1023 head B
    c_ = np.arange(8)[:, None]
    col = np.arange(512)[None, :]
    bd = (col // 64 == c_).astype(np.float32)
    bdmask = np.concatenate([bd, bd], axis=1)
    bf = ml_dtypes.bfloat16
    return {
        "ident": ident,
        "maskqk": maskqk,
        "meanm": meanm.astype(bf),
        "triexc": triexc.astype(bf),
        "ones18": ones18.astype(bf),
        "ones81": ones81.astype(bf),
        "ones11": ones11.astype(bf),
        "onesD": onesD.astype(bf),
        "bdmask": bdmask,
    }


def _in_maps(x, W_qkv, W_out):
    bf = ml_dtypes.bfloat16
    consts = _consts()
    maps = []
    for c in range(N_CORES):
        b, hh = c // 2, c % 2
        heads = list(range(hh * HPC, (hh + 1) * HPC))
        xT = np.ascontiguousarray(x[b].T).astype(bf)
        qcols = np.concatenate(
            [W_qkv[:, 0 * DIM + h * D:(0 * DIM) + (h + 1) * D] for h in heads], axis=1)
        kcols = np.concatenate(
            [W_qkv[:, 1 * DIM + h * D:1 * DIM + (h + 1) * D] for h in heads], axis=1)
        vcols = np.concatenate(
            [W_qkv[:, 2 * DIM + h * D:2 * DIM + (h + 1) * D] for h in heads], axis=1)
        wqk = np.concatenate([qcols * SCALE, kcols], axis=1).astype(bf)
        wv = vcols.astype(bf)
        wout = np.concatenate([W_out[h * D:(h + 1) * D, :] for h in heads],
                              axis=0).astype(bf)
        m = {"xT": xT, "wqk": np.ascontiguousarray(wqk),
             "wv": np.ascontiguousarray(wv), "wout": np.ascontiguousarray(wout)}
        m.update(consts)
        maps.append(m)
    return maps


def kernel(x, W_qkv, W_out, _trace=False):
    if "nc" not in _cache:
        _cache["nc"] = _build()
    nc = _cache["nc"]
    maps = _in_maps(np.asarray(x, np.float32), np.asarray(W_qkv, np.float32),
                    np.asarray(W_out, np.float32))
    res = run_bass_kernel_spmd(nc, maps, core_ids=list(range(N_CORES)),
                               trace=_trace)
    _cache["last_result"] = res
    out = np.empty((B, N, DIM), np.float32)
    for b in range(B):
        out[b] = res.results[2 * b]["out"] + res.results[2 * b + 1]["out"]
    return out
